# revision 9
# baseline (speedup 1.0000x reference)
"""KAN layer (B-spline + silu) Trainium2 kernel, 8-way tensor-parallel.

Math (uniform knot grid):
  Truncated-power features S_i(v) = relu(v - i)^3, v = (x - t0)/h, i = 0..14,
  are computed in f32 on the scalar/vector engines (relu -> square -> mul).
  A small banded f32 matmul on the PE ("combine") turns them into the local
  B-spline basis  B_f = sum_{r=0..4} w5[r] * S_{f+r},  f = 0..10  (w5 =
  [1,-4,6,-4,1]/6).  The combine must run in f32: the truncated powers (up
  to ~2000) cancel down to B <= 0.67.  Its output is post-cancellation, so
  it is cast to fp16, and the main matmul runs fully in fp16 (1 PE
  cycle/row instead of 4 for f32):
      out[n, j*256+q] = sum_f B_f(v[n,j]) * Cw[f, j*256+q]
                        + silu(x[n,j]) * W[j*256+q],   Cw = C * W.
  fp16 scaling: weights are stored as 32*Cw / 32*W (lifting them out of the
  fp16 subnormal range) and the basis as B/32, silu/32 — the f32 PSUM result
  is the unscaled output.  The output is written to HBM in fp16 (halving the
  HBM-write floor, which dominates) and widened to f32 on the host.

Sharding: core s owns j in [32s, 32s+32) (columns [8192s, 8192(s+1)) of the
flattened output).  Per core, j's are grouped into 4 octets of 8; within an
octet, j-pairs map to the 4 PE row groups.  Row layout per 32-row group:
  S tile (f32):  [15 S(j_a), 15 S(j_b), silu'(j_a), silu'(j_b)]
  B tile (fp16): [11 B'(j_a), 11 B'(j_b), silu'(j_a), silu'(j_b), 8 zeros]
The combine matmul (K=32 -> M=32, tile_position (32r,32r), silu rows passed
through, last 8 out-cols zero) and the main matmul (K=32, rhs rows 24..31
zero, tile_position (32r,0)) both use full 32-row groups.

Scheduling: emission is a per-octet wavefront so each in-order engine queue
matches data-readiness: input DMAs ride the scalar queue, output DMAs own
the sync queue (one per (octet, chunk) so the first store issues ~10us in),
and PSUM evacuation alternates scalar/vector.
"""

import numpy as np

import concourse.bass as bass
import concourse.bacc as bacc
import concourse.tile as tile
from concourse import mybir
from concourse.bass_utils import run_bass_kernel_spmd

N = 2048          # batch
N_IN = 256
N_OUT = 256
NCORES = 8
JPC = N_IN // NCORES      # 32 j per core
NOCT = JPC // 8           # 4 octets of 8 j's
NCHUNK = N // 128         # 16 n-chunks
NQ = N // 512             # 4 combine pieces along n
F32 = mybir.dt.float32
F16 = mybir.dt.float16
WSCALE = 32.0             # fp16 weight scale (basis/silu carry 1/32)


def _build_bass(scale_val: float):
    nc = bacc.Bacc(trn_type="TRN2")

    xrep = nc.dram_tensor("xrep", [NOCT, 128, N], F16, kind="ExternalInput")
    biasv = nc.dram_tensor("biasv", [128, 1], F32, kind="ExternalInput")
    w5b = nc.dram_tensor("w5b", [128, 32], F32, kind="ExternalInput")
    rhsbd = nc.dram_tensor("rhsbd", [128, NOCT * 512], F16, kind="ExternalInput")
    siluT = nc.dram_tensor("siluT", [JPC, N], F32, kind="ExternalInput")
    out = nc.dram_tensor("out", [N, JPC * N_OUT], F16, kind="ExternalOutput")

    with tile.TileContext(nc) as tc:
        with (
            tc.tile_pool(name="consts", bufs=1) as consts,
            tc.tile_pool(name="xin", bufs=4) as xin,
            tc.tile_pool(name="chain", bufs=2) as chain,
            tc.tile_pool(name="ss", bufs=1) as sspool,
            tc.tile_pool(name="bsb", bufs=1) as bpool,
            tc.tile_pool(name="stage", bufs=4) as stage_pool,
            tc.tile_pool(name="psumB", bufs=2, space="PSUM") as psumB,
            tc.tile_pool(name="psum", bufs=3, space="PSUM") as psum_pool,
        ):
            # Input loads ride the scalar DMA queue so the sync queue is
            # free for output stores from the start.
            rhs_sb = consts.tile([128, NOCT * 512], F16, name="rhs_sb")
            nc.scalar.dma_start(out=rhs_sb, in_=rhsbd[:, :])
            w5b_sb = consts.tile([128, 32], F32, name="w5b_sb")
            nc.scalar.dma_start(out=w5b_sb, in_=w5b[:, :])
            bias_sb = consts.tile([128, 1], F32, name="bias_sb")
            nc.scalar.dma_start(out=bias_sb, in_=biasv[:, :])
            xr_tiles = []
            for o in range(NOCT):
                xr = xin.tile([128, N], F16, tag=f"xr{o}", name=f"xr{o}")
                nc.scalar.dma_start(out=xr, in_=xrep[o])
                xr_tiles.append(xr)

            cnt = 0
            for o in range(NOCT):
                # --- chain: truncated powers S (f32) ---
                t1 = chain.tile([128, N], F32, tag="t1", name=f"t1_{o}")
                nc.scalar.activation(
                    t1, xr_tiles[o], mybir.ActivationFunctionType.Relu,
                    bias=bias_sb[:, 0:1], scale=scale_val,
                )
                t2 = chain.tile([128, N], F32, tag="t2", name=f"t2_{o}")
                nc.scalar.square(t2, t1)
                ss = sspool.tile([128, N], F32, tag=f"ss{o}", name=f"ss{o}")
                nc.vector.tensor_mul(ss, t1, t2)
                # silu rows (chain wrote exact zeros there; scatter after mul)
                for r in range(4):
                    nc.scalar.dma_start(
                        out=ss[32 * r + 30 : 32 * r + 32, :],
                        in_=siluT[8 * o + 2 * r : 8 * o + 2 * r + 2, :],
                    )

                # --- combine: S -> B' (fp16), banded f32 PE matmul ---
                bsb = bpool.tile([128, N], F16, tag=f"b{o}", name=f"b{o}")
                for q in range(NQ):
                    bps = psumB.tile([128, 512], F32, tag="bps",
                                     name=f"bps{o}_{q}")
                    for r in range(4):
                        nc.tensor.matmul(
                            bps[32 * r : 32 * r + 32, :],
                            lhsT=w5b_sb[32 * r : 32 * r + 32, :],
                            rhs=ss[32 * r : 32 * r + 32,
                                   512 * q : 512 * (q + 1)],
                            start=True,
                            stop=True,
                            tile_position=(32 * r, 32 * r),
                        )
                    dst = bsb[:, 512 * q : 512 * (q + 1)]
                    if q % 2 == 0:
                        nc.vector.tensor_scalar_mul(dst, bps, 1.0)
                    else:
                        nc.scalar.copy(dst, bps)

                # --- main fp16 matmuls, PSUM evacuation, per-piece store ---
                for c in range(NCHUNK):
                    st = stage_pool.tile([128, 2048], F16, tag="st",
                                         name=f"st{o}_{c}")
                    for rp in range(2):  # row-group pairs (2rp, 2rp+1)
                        ps = psum_pool.tile([128, 1024], F32, tag="ps",
                                            name=f"ps{o}_{c}_{rp}")
                        for rr in range(2):
                            r = 2 * rp + rr
                            nc.tensor.matmul(
                                ps[:, 512 * rr : 512 * (rr + 1)],
                                lhsT=bsb[32 * r : 32 * r + 32,
                                         128 * c : 128 * (c + 1)],
                                rhs=rhs_sb[32 * r : 32 * r + 32,
                                           512 * o : 512 * (o + 1)],
                                start=True,
                                stop=True,
                                tile_position=(32 * r, 0),
                            )
                        dst = st[:, 1024 * rp : 1024 * (rp + 1)]
                        if cnt % 2 == 0:
                            nc.vector.tensor_scalar_mul(dst, ps, 1.0)
                        else:
                            nc.scalar.copy(dst, ps)
                        cnt += 1
                    nc.sync.dma_start(
                        out=out[128 * c : 128 * (c + 1),
                                2048 * o : 2048 * (o + 1)],
                        in_=st,
                    )

    nc.compile()
    return nc


def _host_prep(x, C, W, grid):
    """Build per-core input maps."""
    t0 = np.float64(grid[0, 0])
    h = np.float64(grid[0, 1] - grid[0, 0])
    w5 = np.array([1.0, -4.0, 6.0, -4.0, 1.0], np.float64) / 6.0

    # Banded combine weights (f32): B'_f = sum_r (w5[r]/32) S_{f+r} for both
    # j's of the pair, silu pass-through rows 30/31 -> 22/23, cols 24..31 = 0.
    w5b1 = np.zeros((32, 32), np.float32)
    for f in range(11):
        for r in range(5):
            w5b1[f + r, f] = np.float32(w5[r] / WSCALE)
            w5b1[15 + f + r, 11 + f] = np.float32(w5[r] / WSCALE)
    w5b1[30, 22] = 1.0
    w5b1[31, 23] = 1.0
    w5b = np.ascontiguousarray(np.tile(w5b1, (4, 1)))  # same block per row group

    Cw32 = (C.astype(np.float64) * W.astype(np.float64) * WSCALE).astype(np.float16)
    W32 = (W.astype(np.float64) * WSCALE).astype(np.float16)

    xd = x.astype(np.float64)
    silu_p = (xd / (1.0 + np.exp(-xd)) / WSCALE).astype(np.float32)  # silu/32

    # S-tile partition layout within a 32-row group:
    #   s in [0,15)  -> S_i of j_a (i = s)
    #   s in [15,30) -> S_i of j_b (i = s - 15)
    #   s = 30/31    -> silu'(j_a)/silu'(j_b) (scatter; relu bias -64 ->
    #                   the chain writes exact zeros there first)
    s_idx = np.arange(128) % 32
    feat_i = np.where(s_idx < 15, s_idx, np.where(s_idx < 30, s_idx - 15, 0))
    which_b = np.where(s_idx < 15, 0, np.where(s_idx < 30, 1, s_idx - 30))
    biasv = np.where(
        s_idx < 30, -t0 / h - feat_i, -64.0
    ).astype(np.float32).reshape(128, 1)
    scale_val = float(np.float32(1.0 / h))

    x16 = x.astype(np.float16)
    in_maps = []
    for s in range(NCORES):
        jb = JPC * s
        xt = np.ascontiguousarray(x16[:, jb : jb + JPC].T)    # (32, N) fp16
        xrep = np.empty((NOCT, 128, N), np.float16)
        rgrp = np.arange(128) // 32
        for o in range(NOCT):
            jloc = 8 * o + 2 * rgrp + which_b
            xrep[o] = xt[jloc]
        silu_t = np.ascontiguousarray(silu_p[:, jb : jb + JPC].T)  # (32, N) f32

        # B-tile row layout per group: [11 B'a, 11 B'b, silu'a, silu'b, 8 pad]
        rhsbd = np.zeros((128, NOCT * 512), np.float16)
        for o in range(NOCT):
            for rr in range(4):
                ja = (jb + 8 * o + 2 * rr) * N_OUT
                jbc = (jb + 8 * o + 2 * rr + 1) * N_OUT
                base = 32 * rr
                rhsbd[base : base + 11, 512 * o : 512 * o + 256] = \
                    Cw32[:, ja : ja + 256]
                rhsbd[base + 11 : base + 22, 512 * o + 256 : 512 * o + 512] = \
                    Cw32[:, jbc : jbc + 256]
                rhsbd[base + 22, 512 * o : 512 * o + 256] = W32[0, ja : ja + 256]
                rhsbd[base + 23, 512 * o + 256 : 512 * o + 512] = \
                    W32[0, jbc : jbc + 256]
        in_maps.append({
            "xrep": np.ascontiguousarray(xrep),
            "biasv": biasv,
            "w5b": w5b,
            "rhsbd": np.ascontiguousarray(rhsbd),
            "siluT": silu_t,
        })
    return in_maps, scale_val


def kernel(x, C, W, grid):
    in_maps, scale_val = _host_prep(
        np.asarray(x, np.float32), np.asarray(C, np.float32),
        np.asarray(W, np.float32), np.asarray(grid, np.float32),
    )
    nc = _build_bass(scale_val)
    res = run_bass_kernel_spmd(nc, in_maps, core_ids=list(range(NCORES)))
    return np.ascontiguousarray(
        np.concatenate(
            [r["out"].astype(np.float32) for r in res.results], axis=1)
    )


if __name__ == "__main__":
    rng = np.random.default_rng(0)
    x = rng.standard_normal((N, N_IN), dtype=np.float32)
    C = rng.standard_normal((11, N_IN * N_OUT), dtype=np.float32) * 0.005
    W = rng.standard_normal((1, N_IN * N_OUT), dtype=np.float32) * 0.005
    knots = -5.25 + 0.75 * np.arange(15, dtype=np.float32)
    grid = np.tile(knots, (N_IN, 1))
    out = kernel(x, C, W, grid)
    print("kernel out:", out.shape, out.dtype, float(np.abs(out).mean()))


# revision 12
# speedup vs baseline: 1.2415x; 1.2415x over previous
"""KAN layer (B-spline + silu) Trainium2 kernel, 8-way tensor-parallel.

Math (uniform knot grid):
  Truncated-power features S_i(v) = relu(v - i)^3, v = (x - t0)/h, i = 0..14,
  are computed in f32 on the scalar/vector engines (relu -> square -> mul).
  A small banded f32 matmul on the PE ("combine") turns them into the local
  B-spline basis  B_f = sum_{r=0..4} w5[r] * S_{f+r},  f = 0..10  (w5 =
  [1,-4,6,-4,1]/6).  The combine must run in f32: the truncated powers (up
  to ~2000) cancel down to B <= 0.67.  Its output is post-cancellation, so
  it is cast to fp16, and the main matmul runs fully in fp16 (1 PE
  cycle/row instead of 4 for f32):
      out[n, j*256+q] = sum_f B_f(v[n,j]) * Cw[f, j*256+q]
                        + silu(x[n,j]) * W[j*256+q],   Cw = C * W.
  fp16 scaling: weights are stored as 32*Cw / 32*W (lifting them out of the
  fp16 subnormal range) and the basis as B/32, silu/32 — the f32 PSUM result
  is the unscaled output.  The output is written to HBM in fp16 (halving the
  HBM-write floor, which dominates) and widened to f32 on the host.

Sharding: core s owns j in [32s, 32s+32) (columns [8192s, 8192(s+1)) of the
flattened output).  Per core, j's are grouped into 4 octets of 8; within an
octet, j-pairs map to the 4 PE row groups.  Row layout per 32-row group:
  S tile (f32):  [15 S(j_a), 15 S(j_b), silu'(j_a), silu'(j_b)]
  B tile (fp16): [11 B'(j_a), 11 B'(j_b), silu'(j_a), silu'(j_b), 8 zeros]
The combine matmul (K=32 -> M=32, tile_position (32r,32r), silu rows passed
through, last 8 out-cols zero) and the main matmul (K=32, rhs rows 24..31
zero, tile_position (32r,0)) both use full 32-row groups.

Scheduling: emission is a per-octet wavefront so each in-order engine queue
matches data-readiness: input DMAs ride the scalar queue, output DMAs own
the sync queue (one per (octet, chunk) so the first store issues ~10us in),
and PSUM evacuation alternates scalar/vector.
"""

import numpy as np

import concourse.bass as bass
import concourse.bacc as bacc
import concourse.tile as tile
from concourse import mybir
from concourse.bass_utils import run_bass_kernel_spmd

N = 2048          # batch
N_IN = 256
N_OUT = 256
NCORES = 8
JPC = N_IN // NCORES      # 32 j per core
NOCT = JPC // 8           # 4 octets of 8 j's
NCHUNK = N // 128         # 16 n-chunks
NQ = N // 512             # 4 combine pieces along n
F32 = mybir.dt.float32
F16 = mybir.dt.float16
WSCALE = 32.0             # fp16 weight scale (basis/silu carry 1/32)


def _build_bass(scale_val: float):
    nc = bacc.Bacc(trn_type="TRN2")

    xrep = nc.dram_tensor("xrep", [NOCT, 128, N], F16, kind="ExternalInput")
    biasv = nc.dram_tensor("biasv", [128, 1], F32, kind="ExternalInput")
    w5b = nc.dram_tensor("w5b", [128, 32], F32, kind="ExternalInput")
    rhsbd = nc.dram_tensor("rhsbd", [128, NOCT * 512], F16, kind="ExternalInput")
    siluT = nc.dram_tensor("siluT", [JPC, N], F32, kind="ExternalInput")
    out = nc.dram_tensor("out", [N, JPC * N_OUT], F16, kind="ExternalOutput")

    with tile.TileContext(nc) as tc:
        with (
            tc.tile_pool(name="consts", bufs=1) as consts,
            tc.tile_pool(name="xin", bufs=4) as xin,
            tc.tile_pool(name="chain", bufs=2) as chain,
            tc.tile_pool(name="ss", bufs=1) as sspool,
            tc.tile_pool(name="bsb", bufs=1) as bpool,
            tc.tile_pool(name="stage", bufs=4) as stage_pool,
            tc.tile_pool(name="psum", bufs=4, space="PSUM") as psum_pool,
        ):
            # Input loads ride the scalar DMA queue so the sync queue is
            # free for output stores from the start.
            rhs_sb = consts.tile([128, NOCT * 512], F16, name="rhs_sb")
            nc.scalar.dma_start(out=rhs_sb, in_=rhsbd[:, :])
            w5b_sb = consts.tile([128, 32], F32, name="w5b_sb")
            nc.scalar.dma_start(out=w5b_sb, in_=w5b[:, :])
            bias_sb = consts.tile([128, 1], F32, name="bias_sb")
            nc.scalar.dma_start(out=bias_sb, in_=biasv[:, :])
            xr_tiles = []
            for o in range(NOCT):
                xr = xin.tile([128, N], F16, tag=f"xr{o}", name=f"xr{o}")
                nc.scalar.dma_start(out=xr, in_=xrep[o])
                xr_tiles.append(xr)

            cnt = 0
            for o in range(NOCT):
                # --- chain: truncated powers S (f32) ---
                t1 = chain.tile([128, N], F32, tag="t1", name=f"t1_{o}")
                nc.scalar.activation(
                    t1, xr_tiles[o], mybir.ActivationFunctionType.Relu,
                    bias=bias_sb[:, 0:1], scale=scale_val,
                )
                t2 = chain.tile([128, N], F32, tag="t2", name=f"t2_{o}")
                nc.scalar.square(t2, t1)
                ss = sspool.tile([128, N], F32, tag=f"ss{o}", name=f"ss{o}")
                nc.vector.tensor_mul(ss, t1, t2)
                # silu rows (chain wrote exact zeros there; scatter after mul)
                for r in range(4):
                    nc.scalar.dma_start(
                        out=ss[32 * r + 30 : 32 * r + 32, :],
                        in_=siluT[8 * o + 2 * r : 8 * o + 2 * r + 2, :],
                    )

                # --- combine: S -> B' (fp16), banded f32 PE matmul ---
                bsb = bpool.tile([128, N], F16, tag=f"b{o}", name=f"b{o}")
                for q in range(NQ):
                    bpt = psum_pool.tile([128, 1024], F32, tag="ps",
                                         name=f"bps{o}_{q}")
                    bps = bpt[:, 0:512]
                    for r in range(4):
                        nc.tensor.matmul(
                            bps[32 * r : 32 * r + 32, :],
                            lhsT=w5b_sb[32 * r : 32 * r + 32, :],
                            rhs=ss[32 * r : 32 * r + 32,
                                   512 * q : 512 * (q + 1)],
                            start=True,
                            stop=True,
                            tile_position=(32 * r, 32 * r),
                        )
                    dst = bsb[:, 512 * q : 512 * (q + 1)]
                    if q % 2 == 0:
                        nc.vector.tensor_scalar_mul(dst, bps, 1.0)
                    else:
                        nc.scalar.copy(dst, bps)

                # --- main fp16 matmuls, PSUM evacuation, per-piece store ---
                for c in range(NCHUNK):
                    st = stage_pool.tile([128, 2048], F16, tag="st",
                                         name=f"st{o}_{c}")
                    for rp in range(2):  # row-group pairs (2rp, 2rp+1)
                        ps = psum_pool.tile([128, 1024], F32, tag="ps",
                                            name=f"ps{o}_{c}_{rp}")
                        for rr in range(2):
                            r = 2 * rp + rr
                            nc.tensor.matmul(
                                ps[:, 512 * rr : 512 * (rr + 1)],
                                lhsT=bsb[32 * r : 32 * r + 32,
                                         128 * c : 128 * (c + 1)],
                                rhs=rhs_sb[32 * r : 32 * r + 32,
                                           512 * o : 512 * (o + 1)],
                                start=True,
                                stop=True,
                                tile_position=(32 * r, 0),
                            )
                        dst = st[:, 1024 * rp : 1024 * (rp + 1)]
                        # scalar also runs the chain: give it 7 of 16 copies
                        # per octet (vector 9) by skewing the alternation.
                        if cnt % 16 in (0, 2, 4, 6, 8, 10, 12, 13, 14):
                            nc.vector.tensor_scalar_mul(dst, ps, 1.0)
                        else:
                            nc.scalar.copy(dst, ps)
                        cnt += 1
                    nc.sync.dma_start(
                        out=out[128 * c : 128 * (c + 1),
                                2048 * o : 2048 * (o + 1)],
                        in_=st,
                    )

    nc.compile()
    return nc


def _host_prep(x, C, W, grid):
    """Build per-core input maps."""
    t0 = np.float64(grid[0, 0])
    h = np.float64(grid[0, 1] - grid[0, 0])
    w5 = np.array([1.0, -4.0, 6.0, -4.0, 1.0], np.float64) / 6.0

    # Banded combine weights (f32): B'_f = sum_r (w5[r]/32) S_{f+r} for both
    # j's of the pair, silu pass-through rows 30/31 -> 22/23, cols 24..31 = 0.
    w5b1 = np.zeros((32, 32), np.float32)
    for f in range(11):
        for r in range(5):
            w5b1[f + r, f] = np.float32(w5[r] / WSCALE)
            w5b1[15 + f + r, 11 + f] = np.float32(w5[r] / WSCALE)
    w5b1[30, 22] = 1.0
    w5b1[31, 23] = 1.0
    w5b = np.ascontiguousarray(np.tile(w5b1, (4, 1)))  # same block per row group

    Cw32 = (C.astype(np.float64) * W.astype(np.float64) * WSCALE).astype(np.float16)
    W32 = (W.astype(np.float64) * WSCALE).astype(np.float16)

    xd = x.astype(np.float64)
    silu_p = (xd / (1.0 + np.exp(-xd)) / WSCALE).astype(np.float32)  # silu/32

    # S-tile partition layout within a 32-row group:
    #   s in [0,15)  -> S_i of j_a (i = s)
    #   s in [15,30) -> S_i of j_b (i = s - 15)
    #   s = 30/31    -> silu'(j_a)/silu'(j_b) (scatter; relu bias -64 ->
    #                   the chain writes exact zeros there first)
    s_idx = np.arange(128) % 32
    feat_i = np.where(s_idx < 15, s_idx, np.where(s_idx < 30, s_idx - 15, 0))
    which_b = np.where(s_idx < 15, 0, np.where(s_idx < 30, 1, s_idx - 30))
    biasv = np.where(
        s_idx < 30, -t0 / h - feat_i, -64.0
    ).astype(np.float32).reshape(128, 1)
    scale_val = float(np.float32(1.0 / h))

    x16 = x.astype(np.float16)
    in_maps = []
    for s in range(NCORES):
        jb = JPC * s
        xt = np.ascontiguousarray(x16[:, jb : jb + JPC].T)    # (32, N) fp16
        xrep = np.empty((NOCT, 128, N), np.float16)
        rgrp = np.arange(128) // 32
        for o in range(NOCT):
            jloc = 8 * o + 2 * rgrp + which_b
            xrep[o] = xt[jloc]
        silu_t = np.ascontiguousarray(silu_p[:, jb : jb + JPC].T)  # (32, N) f32

        # B-tile row layout per group: [11 B'a, 11 B'b, silu'a, silu'b, 8 pad]
        rhsbd = np.zeros((128, NOCT * 512), np.float16)
        for o in range(NOCT):
            for rr in range(4):
                ja = (jb + 8 * o + 2 * rr) * N_OUT
                jbc = (jb + 8 * o + 2 * rr + 1) * N_OUT
                base = 32 * rr
                rhsbd[base : base + 11, 512 * o : 512 * o + 256] = \
                    Cw32[:, ja : ja + 256]
                rhsbd[base + 11 : base + 22, 512 * o + 256 : 512 * o + 512] = \
                    Cw32[:, jbc : jbc + 256]
                rhsbd[base + 22, 512 * o : 512 * o + 256] = W32[0, ja : ja + 256]
                rhsbd[base + 23, 512 * o + 256 : 512 * o + 512] = \
                    W32[0, jbc : jbc + 256]
        in_maps.append({
            "xrep": np.ascontiguousarray(xrep),
            "biasv": biasv,
            "w5b": w5b,
            "rhsbd": np.ascontiguousarray(rhsbd),
            "siluT": silu_t,
        })
    return in_maps, scale_val


def kernel(x, C, W, grid):
    in_maps, scale_val = _host_prep(
        np.asarray(x, np.float32), np.asarray(C, np.float32),
        np.asarray(W, np.float32), np.asarray(grid, np.float32),
    )
    nc = _build_bass(scale_val)
    res = run_bass_kernel_spmd(nc, in_maps, core_ids=list(range(NCORES)))
    return np.ascontiguousarray(
        np.concatenate(
            [r["out"].astype(np.float32) for r in res.results], axis=1)
    )


if __name__ == "__main__":
    rng = np.random.default_rng(0)
    x = rng.standard_normal((N, N_IN), dtype=np.float32)
    C = rng.standard_normal((11, N_IN * N_OUT), dtype=np.float32) * 0.005
    W = rng.standard_normal((1, N_IN * N_OUT), dtype=np.float32) * 0.005
    knots = -5.25 + 0.75 * np.arange(15, dtype=np.float32)
    grid = np.tile(knots, (N_IN, 1))
    out = kernel(x, C, W, grid)
    print("kernel out:", out.shape, out.dtype, float(np.abs(out).mean()))
